# revision 29
# baseline (speedup 1.0000x reference)
"""KBertGATEnricher Trainium2 kernel.

Sharding: data-parallel over batch (8 batches -> 8 cores) end to end.
Each core runs embedding-LN + GAT for its batch, then the full-vocab
output Linear + log_softmax for its 256 tokens, streaming out_W^T from
HBM under the matmul. No collectives: the concat features stay local and
the softmax normalizer is a per-token (local) reduction.

Self-contained: hardcodes all shapes; only imports the system-installed
concourse runtime.
"""

import math
import os
import sys

sys.path.insert(0, "/opt/trn_rl_repo")

import numpy as np

from concourse import bass, bacc, mybir, tile
from concourse.bass_utils import run_bass_kernel_spmd

F32 = mybir.dt.float32
F16 = mybir.dt.float16
U8 = mybir.dt.uint8

B, N, D, H, F, V = 8, 256, 768, 4, 128, 30522
D2 = H * F        # 512 concat feature dim
NCORES = 8
VPAD = 30528       # padded vocab (6 zero cols)
NPADC = VPAD - V
LN_EPS = 1e-12
ALPHA = 0.01       # leaky relu slope
MASK_NEG = -50.0   # masked attention logit
NKT = D // 128     # 6 hidden k-tiles for the GAT matmuls
NM = N // 128      # 2 token m-tiles per core
# vocab chunks: [128,1024] psum tiles (2 banks); 30528 = 29*1024 + 832
CHUNKS = [(c0, min(1024, VPAD - c0)) for c0 in range(0, VPAD, 1024)]
NVC = len(CHUNKS)

AX = mybir.AxisListType
AF = mybir.ActivationFunctionType
OP = mybir.AluOpType

_NC_CACHE = {}


def _build(with_ln_b: bool, with_out_b: bool):
    """Build the SPMD Bass program (identical on all 8 cores)."""
    nc = bacc.Bacc(
        "TRN2",
        target_bir_lowering=False,
        debug=False,
        enable_asserts=False,
        num_devices=NCORES,
    )

    # ---- per-core I/O --------------------------------------------------
    xpre = nc.dram_tensor("xpre", [N, D], F32, kind="ExternalInput").ap()
    maskt = nc.dram_tensor("maskt", [N, N], U8, kind="ExternalInput").ap()
    wg = nc.dram_tensor("wg", [D, D2], F16, kind="ExternalInput").ap()
    cpk = nc.dram_tensor("cpk", [D, 2 * H], F16, kind="ExternalInput").ap()
    wst = nc.dram_tensor("wst", [4, 128, VPAD], F16, kind="ExternalInput").ap()
    if with_ln_b:
        brw = nc.dram_tensor("brw", [1, D2], F16, kind="ExternalInput").ap()
        brc = nc.dram_tensor("brc", [1, 2 * H], F16, kind="ExternalInput").ap()
    if with_out_b:
        bvoc = nc.dram_tensor("bvoc", [1, VPAD], F16, kind="ExternalInput").ap()
    out = nc.dram_tensor("out", [N, VPAD], F16, kind="ExternalOutput").ap()

    with tile.TileContext(nc) as tc:
        with (
            tc.tile_pool(name="catf_pool", bufs=1) as catf_pool,
            tc.tile_pool(name="bias_pool", bufs=1) as bias_pool,
        ):
            # local concat features [feature-k, token] per head (== k-tile)
            catf = [
                catf_pool.tile([128, N], F16, tag=f"catf{kt}", name=f"catf{kt}") for kt in range(H)
            ]
            if with_out_b:
                bvoc_sb = bias_pool.tile([1, VPAD], F16, tag="bvoc")
                nc.sync.dma_start(out=bvoc_sb[:], in_=bvoc[:, :])
                ones1v = bias_pool.tile([1, 128], F16, tag="ones1v")
                nc.vector.memset(ones1v[:], 1.0)

            # ==== phase A: embedding LN + GAT (own batch) ==============
            with (
                tc.tile_pool(name="pa", bufs=1) as pa,
                tc.tile_pool(name="pa_tmp", bufs=4) as pa_tmp,
                tc.tile_pool(name="ps_tr", bufs=2, space="PSUM") as ps_tr,
                tc.tile_pool(name="ps_wh", bufs=2, space="PSUM") as ps_wh,
                tc.tile_pool(name="ps_s12", bufs=1, space="PSUM") as ps_s12,
                tc.tile_pool(name="ps_att", bufs=2, space="PSUM") as ps_att,
                tc.tile_pool(name="ps_hp", bufs=1, space="PSUM") as ps_hp,
            ):
                idw = pa.tile([128, 128], F16, tag="idw")
                bass_masks_identity(nc, idw[:])
                negt = pa.tile([128, N], F32, tag="negt")
                nc.vector.memset(negt[:], MASK_NEG)
                ones_nf = pa.tile([1, N], F16, tag="ones_nf")
                nc.vector.memset(ones_nf[:], 1.0)
                eps_sb = pa.tile([128, 1], F32, tag="eps_sb")
                nc.vector.memset(eps_sb[:], LN_EPS)
                if with_ln_b:
                    ones1h = pa.tile([1, 128], F16, tag="ones1h")
                    nc.vector.memset(ones1h[:], 1.0)
                    ones_n = pa.tile([1, N], F16, tag="ones_n")
                    nc.vector.memset(ones_n[:], 1.0)
                    brw_sb = pa.tile([1, D2], F16, tag="brw")
                    nc.sync.dma_start(out=brw_sb[:], in_=brw[:, :])
                    brc_sb = pa.tile([1, 2 * H], F16, tag="brc")
                    nc.sync.dma_start(out=brc_sb[:], in_=brc[:, :])

                # ---- LayerNorm first: xpre DMA must not queue behind
                # the big wst loads (phase B throttles those via its pool)
                xn_sb = [pa.tile([128, D], F16, tag=f"xn{m}", name=f"xn{m}") for m in range(2)]
                xp_t = []
                for m in range(2):
                    xp = pa_tmp.tile([128, D], F32, tag="xp")
                    nc.sync.dma_start(
                        out=xp[:], in_=xpre[m * 128 : (m + 1) * 128, :]
                    )
                    xp_t.append(xp)
                wg_sb = [
                    pa.tile([128, D2], F16, tag=f"wg{kt}", name=f"wg{kt}") for kt in range(NKT)
                ]
                cpk_sb = [
                    pa.tile([128, 2 * H], F16, tag=f"cpk{kt}", name=f"cpk{kt}") for kt in range(NKT)
                ]
                for kt in range(NKT):
                    nc.sync.dma_start(
                        out=wg_sb[kt][:], in_=wg[kt * 128 : (kt + 1) * 128, :]
                    )
                    nc.sync.dma_start(
                        out=cpk_sb[kt][:], in_=cpk[kt * 128 : (kt + 1) * 128, :]
                    )
                mask_sb = [pa.tile([128, N], U8, tag=f"mask{j}", name=f"mask{j}") for j in range(2)]
                for jt in range(2):
                    nc.sync.dma_start(
                        out=mask_sb[jt][:], in_=maskt[jt * 128 : (jt + 1) * 128, :]
                    )
                for m in range(2):
                    xp = xp_t[m]
                    nmu = pa_tmp.tile([128, 1], F32, tag="nmu")
                    nc.vector.tensor_reduce(
                        out=nmu[:], in_=xp[:], axis=AX.X, op=OP.add, negate=True
                    )
                    nc.vector.tensor_scalar_mul(nmu[:], nmu[:], 1.0 / D)
                    xc = pa_tmp.tile([128, D], F32, tag="xc")
                    nc.vector.tensor_scalar_add(xc[:], xp[:], nmu[:, 0:1])
                    sq = pa_tmp.tile([128, D], F32, tag="sq")
                    ssum = pa_tmp.tile([128, 1], F32, tag="ssum")
                    nc.scalar.activation(
                        sq[:], xc[:], AF.Square, accum_out=ssum[:, 0:1]
                    )
                    sd = pa_tmp.tile([128, 1], F32, tag="sd")
                    nc.scalar.activation(
                        sd[:], ssum[:], AF.Sqrt, bias=eps_sb[:, 0:1], scale=1.0 / D
                    )
                    rstd = pa_tmp.tile([128, 1], F32, tag="rstd")
                    nc.vector.reciprocal(rstd[:], sd[:])
                    nc.vector.tensor_scalar_mul(xn_sb[m][:], xc[:], rstd[:, 0:1])

                # ---- transpose xn -> xT[kt] [128 hid, 256 tok] --------
                xt_sb = [pa.tile([128, N], F16, tag=f"xt{kt}", name=f"xt{kt}") for kt in range(NKT)]
                for kt in range(NKT):
                    for m in range(2):
                        ptr = ps_tr.tile([128, 128], F16, tag="ptr")
                        nc.tensor.transpose(
                            ptr[:], xn_sb[m][:, kt * 128 : (kt + 1) * 128], idw[:]
                        )
                        nc.scalar.copy(
                            xt_sb[kt][:, m * 128 : (m + 1) * 128], ptr[:]
                        )

                # ---- fused Wh for all heads: [tokens, 512] ------------
                wh_m = [pa.tile([128, D2], F16, tag=f"whm{m}", name=f"whm{m}") for m in range(2)]
                for m in range(2):
                    pwh = ps_wh.tile([128, D2], F32, tag="pwh")
                    for kt in range(NKT):
                        nc.tensor.matmul(
                            pwh[:],
                            xt_sb[kt][:, m * 128 : (m + 1) * 128],
                            wg_sb[kt][:],
                            start=(kt == 0),
                            stop=(kt == NKT - 1) and not with_ln_b,
                        )
                    if with_ln_b:
                        nc.tensor.matmul(
                            pwh[:], ones1h[:], brw_sb[:], start=False, stop=True
                        )
                    nc.scalar.copy(wh_m[m][:], pwh[:])

                # ---- fused s1/s2 rows: [8 = (h,c), tokens] ------------
                ps12 = ps_s12.tile([2 * H, N], F32, tag="ps12")
                for kt in range(NKT):
                    nc.tensor.matmul(
                        ps12[:],
                        cpk_sb[kt][:],
                        xt_sb[kt][:],
                        start=(kt == 0),
                        stop=(kt == NKT - 1) and not with_ln_b,
                    )
                if with_ln_b:
                    nc.tensor.matmul(
                        ps12[:], brc_sb[:], ones_n[:], start=False, stop=True
                    )
                s12sb = pa.tile([2 * H, N], F16, tag="s12sb")
                nc.scalar.copy(s12sb[:], ps12[:])

                # ---- attention + head output, one head at a time ------
                att = [
                    [pa.tile([128, N], F16, tag=f"att{h}_{m}", name=f"att{h}_{m}") for m in range(2)]
                    for h in range(H)
                ]
                for h in range(H):
                    # rhs rows: [s1(tokens); ones] -- row moves via DMA
                    # (engine ops cannot write at partition offsets != 0)
                    s1r2 = pa_tmp.tile([2, N], F16, tag="s1r2")
                    nc.sync.dma_start(
                        out=s1r2[0:1, :], in_=s12sb[2 * h : 2 * h + 1, :]
                    )
                    nc.sync.dma_start(out=s1r2[1:2, :], in_=ones_nf[0:1, :])
                    for jt in range(2):
                        # lhsT rows: [ones; s2(j-block)]
                        s2st = pa_tmp.tile([2, 128], F16, tag="s2st")
                        nc.sync.dma_start(
                            out=s2st[0:1, :], in_=ones_nf[0:1, 0:128]
                        )
                        nc.sync.dma_start(
                            out=s2st[1:2, :],
                            in_=s12sb[2 * h + 1 : 2 * h + 2, jt * 128 : (jt + 1) * 128],
                        )
                        pet = ps_att.tile([128, N], F32, tag="pet")
                        nc.tensor.matmul(
                            pet[:], s2st[:], s1r2[:], start=True, stop=True
                        )
                        # leakyrelu in 2 ops (only one PSUM input per instr)
                        lr1 = pa_tmp.tile([128, N], F32, tag="lr1")
                        nc.vector.tensor_scalar_mul(lr1[:], pet[:], ALPHA)
                        lr = pa_tmp.tile([128, N], F32, tag="lr")
                        nc.vector.scalar_tensor_tensor(
                            lr[:], lr1[:], 1.0, pet[:], OP.mult, OP.max
                        )
                        nc.vector.copy_predicated(lr[:], mask_sb[jt][:], negt[:])
                        nmax = pa_tmp.tile([128, 1], F32, tag="nmax")
                        nc.vector.tensor_reduce(
                            out=nmax[:], in_=lr[:], axis=AX.X, op=OP.max, negate=True
                        )
                        ex = pa_tmp.tile([128, N], F16, tag="ex")
                        asum = pa_tmp.tile([128, 1], F32, tag="asum")
                        nc.scalar.activation(
                            ex[:], lr[:], AF.Exp, bias=nmax[:, 0:1],
                            accum_out=asum[:, 0:1],
                        )
                        rec = pa_tmp.tile([128, 1], F32, tag="rec")
                        nc.vector.reciprocal(rec[:], asum[:])
                        nc.vector.tensor_scalar_mul(
                            att[h][jt][:], ex[:], rec[:, 0:1]
                        )
                    # hp^T = Wh^T @ att^T, elu -> local catf[h]
                    php = ps_hp.tile([128, N], F32, tag="php")
                    for jt in range(2):
                        nc.tensor.matmul(
                            php[:],
                            wh_m[jt][:, h * 128 : (h + 1) * 128],
                            att[h][jt][:],
                            start=(jt == 0),
                            stop=(jt == 1),
                        )
                    # elu(z) = max(z, min(exp(z)-1, 0))
                    eh = pa_tmp.tile([128, N], F32, tag="eh")
                    nc.scalar.activation(eh[:], php[:], AF.Exp)
                    ddh = pa_tmp.tile([128, N], F16, tag="ddh")
                    nc.vector.tensor_scalar(
                        ddh[:], eh[:], 1.0, 0.0, OP.subtract, OP.min
                    )
                    nc.vector.scalar_tensor_tensor(
                        catf[h][:], php[:], 0.0, ddh[:], OP.add, OP.max
                    )

            # ==== full-vocab output linear + local log_softmax =========
            with (
                tc.tile_pool(name="vp_pool", bufs=1) as vp_pool,
                tc.tile_pool(name="wstream", bufs=3) as wstream,
                tc.tile_pool(name="etmp", bufs=4) as etmp,
                tc.tile_pool(name="stat", bufs=1) as stat,
                tc.tile_pool(name="ps_z", bufs=4, space="PSUM") as ps_z,
            ):
                vp = [
                    vp_pool.tile([128, VPAD], F16, tag=f"vp{m}", name=f"vp{m}") for m in range(NM)
                ]
                # exp sums per 2-chunk group, one tile per m-tile
                NSG = (NVC + 1) // 2
                sums = [
                    stat.tile([128, NSG], F32, tag=f"sums{m}", name=f"sums{m}")
                    for m in range(NM)
                ]
                negone = stat.tile([128, 1], F32, tag="negone")
                nc.vector.memset(negone[:], -1.0)
                pad_sb = stat.tile([128, 1], F32, tag="pad_sb")
                nc.vector.memset(pad_sb[:], NPADC * math.exp(-1.0))

                for ci, (c0, cw) in enumerate(CHUNKS):
                    wc = [
                        wstream.tile([128, 1024], F16, tag=f"wc{kt}", name=f"wc{kt}")
                        for kt in range(4)
                    ]
                    for kt in range(4):
                        nc.sync.dma_start(
                            out=wc[kt][:, 0:cw], in_=wst[kt, :, c0 : c0 + cw]
                        )
                    for m in range(NM):
                        zp = ps_z.tile([128, 1024], F32, tag="z")
                        for s0 in range(0, cw, 512):
                            sw = min(512, cw - s0)
                            for kt in range(4):
                                nc.tensor.matmul(
                                    zp[:, s0 : s0 + sw],
                                    catf[kt][:, m * 128 : (m + 1) * 128],
                                    wc[kt][:, s0 : s0 + sw],
                                    start=(kt == 0),
                                    stop=(kt == 3) and not with_out_b,
                                )
                            if with_out_b:
                                nc.tensor.matmul(
                                    zp[:, s0 : s0 + sw],
                                    ones1v[:],
                                    bvoc_sb[:, c0 + s0 : c0 + s0 + sw],
                                    start=False,
                                    stop=True,
                                )
                        # vp = elu(z) = max(z, min(exp(z)-1, 0))
                        e0 = etmp.tile([128, 1024], F16, tag="e0")
                        nc.scalar.activation(e0[:, 0:cw], zp[:, 0:cw], AF.Exp)
                        dd = etmp.tile([128, 1024], F16, tag="dd")
                        nc.vector.tensor_scalar(
                            dd[:, 0:cw], e0[:, 0:cw], 1.0, 0.0,
                            OP.subtract, OP.min,
                        )
                        nc.vector.scalar_tensor_tensor(
                            vp[m][:, c0 : c0 + cw],
                            zp[:, 0:cw],
                            0.0,
                            dd[:, 0:cw],
                            OP.add,
                            OP.max,
                        )
                    # sum of exp(vp - 1) over each completed 2-chunk group
                    if ci % 2 == 1 or ci == NVC - 1:
                        g0 = (ci // 2) * 2
                        gc0 = CHUNKS[g0][0]
                        gw = c0 + cw - gc0
                        gi = ci // 2
                        for m in range(NM):
                            dum = etmp.tile([128, 2048], F16, tag="dum")
                            nc.scalar.activation(
                                dum[:, 0:gw],
                                vp[m][:, gc0 : gc0 + gw],
                                AF.Exp,
                                bias=negone[:, 0:1],
                                accum_out=sums[m][:, gi : gi + 1],
                            )

                # local logsumexp: pad cols contribute exp(0-1) each;
                # remove, then out = vp - (log(sum) + 1)
                neglog = []
                for m in range(NM):
                    lsum = stat.tile([128, 1], F32, tag=f"lsum{m}", name=f"lsum{m}")
                    nc.vector.tensor_reduce(
                        out=lsum[:], in_=sums[m][:], axis=AX.X, op=OP.add
                    )
                    nc.vector.tensor_scalar_sub(lsum[:], lsum[:], pad_sb[:, 0:1])
                    nl = stat.tile([128, 1], F32, tag=f"neglog{m}", name=f"neglog{m}")
                    nc.scalar.activation(nl[:], lsum[:], AF.Ln)
                    nc.vector.tensor_scalar(
                        nl[:], nl[:], -1.0, 1.0, OP.mult, OP.subtract
                    )
                    neglog.append(nl)
                # piecewise finals: each output DMA piece can start as soon
                # as its slice is normalized instead of after the whole tile
                NP = 4
                pw = VPAD // NP
                for pi in range(NP):
                    p0 = pi * pw
                    for m in range(NM):
                        nc.vector.tensor_scalar_add(
                            vp[m][:, p0 : p0 + pw],
                            vp[m][:, p0 : p0 + pw],
                            neglog[m][:, 0:1],
                        )
                        nc.sync.dma_start(
                            out=out[m * 128 : (m + 1) * 128, p0 : p0 + pw],
                            in_=vp[m][:, p0 : p0 + pw],
                        )

    nc.compile()
    return nc


def bass_masks_identity(nc, ident_ap):
    from concourse import masks

    masks.make_identity(nc, ident_ap)


def _host_prep(inputs):
    """Per-core input maps from full inputs (numpy only)."""
    tok = np.asarray(inputs["token_ids"])
    typ = np.asarray(inputs["type_ids"])
    syn = np.asarray(inputs["synset_ids"])
    hw = np.asarray(inputs["highway"]).astype(bool)
    tok_emb = np.asarray(inputs["tok_emb"], dtype=np.float32)
    type_emb = np.asarray(inputs["type_emb"], dtype=np.float32)
    pos_emb = np.asarray(inputs["pos_emb"], dtype=np.float32)
    ln_g = np.asarray(inputs["ln_g"], dtype=np.float32)
    ln_b = np.asarray(inputs["ln_b"], dtype=np.float32)
    W = np.asarray(inputs["W"], dtype=np.float32)
    a = np.asarray(inputs["a"], dtype=np.float32)
    out_W = np.asarray(inputs["out_W"], dtype=np.float32)
    out_b = np.asarray(inputs["out_b"], dtype=np.float32)

    # embeddings (host gather + add, f32 like the reference)
    x_pre = tok_emb[tok] + type_emb[typ] + pos_emb[:N][None]  # (B,N,D)

    # graph mask (host index logic), transposed to [j, i], 1 = masked-out
    vis = syn[:, :, None] == syn[:, None, :]
    s1m = (typ == 1) & hw
    s3m = (typ == 3) & hw
    d1 = np.isin(typ, [0, 2, 5]) & hw
    d3 = np.isin(typ, [6, 4, 0]) & hw
    vis = vis | (s1m[:, :, None] & d1[:, None, :]) | (s3m[:, :, None] & d3[:, None, :])
    mask = vis & (tok != 0)[:, None, :]  # (B,N,N) over [i,j]
    maskt = (~mask).transpose(0, 2, 1).astype(np.uint8)  # (B,N,N) over [j,i]

    # GAT weights: fold ln_g; head-major [D, H*F] plus c1/c2 contraction cols
    Wg = W * ln_g[None, :, None]  # (H,D,F)
    wg = Wg.transpose(1, 0, 2).reshape(D, D2).astype(np.float16)
    a1, a2 = a[:, :F], a[:, F:]
    c1 = np.einsum("hdf,hf->hd", Wg, a1)  # (H,D)
    c2 = np.einsum("hdf,hf->hd", Wg, a2)
    cpk = np.stack([c1, c2], axis=1).reshape(2 * H, D).T  # (D, 2H) cols (h,c)
    cpk = np.ascontiguousarray(cpk).astype(np.float16)

    with_ln_b = bool(np.any(ln_b != 0.0))
    brw = brc = None
    if with_ln_b:
        brw = (ln_b @ W.transpose(1, 0, 2).reshape(D, D2)).reshape(1, D2)
        brw = brw.astype(np.float16)
        b1 = np.einsum("hdf,hf->hd", W, a1)
        b2 = np.einsum("hdf,hf->hd", W, a2)
        brc = (ln_b @ np.stack([b1, b2], axis=1).reshape(2 * H, D).T).reshape(1, 2 * H)
        brc = brc.astype(np.float16)

    # full padded out_W^T [512, 30528] -> [4, 128, VPAD] (same on all cores)
    wpad = np.zeros((VPAD, D2), dtype=np.float32)
    wpad[:V] = out_W
    wst = np.ascontiguousarray(wpad.T.astype(np.float16).reshape(4, 128, VPAD))
    with_out_b = bool(np.any(out_b != 0.0))
    bvocp = None
    if with_out_b:
        bpad = np.zeros((VPAD,), dtype=np.float32)
        bpad[:V] = out_b
        bvocp = np.ascontiguousarray(bpad.reshape(1, VPAD).astype(np.float16))

    in_maps = []
    for c in range(NCORES):
        m = {
            "xpre": np.ascontiguousarray(x_pre[c]),
            "maskt": np.ascontiguousarray(maskt[c]),
            "wg": wg,
            "cpk": cpk,
            "wst": wst,
        }
        if with_ln_b:
            m["brw"] = brw
            m["brc"] = brc
        if with_out_b:
            m["bvoc"] = bvocp
        in_maps.append(m)
    return in_maps, with_ln_b, with_out_b


def kernel(**inputs) -> np.ndarray:
    in_maps, with_ln_b, with_out_b = _host_prep(inputs)

    key = (with_ln_b, with_out_b)
    if key not in _NC_CACHE:
        _NC_CACHE[key] = _build(with_ln_b, with_out_b)
    nc = _NC_CACHE[key]

    trace = bool(int(os.environ.get("KBERT_TRACE", "0")))
    res = run_bass_kernel_spmd(
        nc, in_maps, core_ids=list(range(NCORES)), trace=trace
    )
    if trace and res.exec_time_ns is not None:
        print(f"HW exec time: {res.exec_time_ns} ns")
        if res.instructions_and_trace is not None:
            print(f"trace: {res.instructions_and_trace[1]}")

    full = np.empty((B, N, V), dtype=np.float32)
    for c in range(NCORES):
        full[c] = res.results[c]["out"][:, :V]
    return full


# revision 30
# speedup vs baseline: 1.1092x; 1.1092x over previous
"""KBertGATEnricher Trainium2 kernel.

Sharding: data-parallel over batch (8 batches -> 8 cores) end to end.
Each core runs embedding-LN + GAT for its batch, then the full-vocab
output Linear + log_softmax for its 256 tokens, streaming out_W^T from
HBM under the matmul. No collectives: the concat features stay local and
the softmax normalizer is a per-token (local) reduction.

Self-contained: hardcodes all shapes; only imports the system-installed
concourse runtime.
"""

import math
import os
import sys

sys.path.insert(0, "/opt/trn_rl_repo")

import numpy as np

from concourse import bass, bacc, mybir, tile
from concourse.bass_utils import run_bass_kernel_spmd

F32 = mybir.dt.float32
F16 = mybir.dt.float16
U8 = mybir.dt.uint8

B, N, D, H, F, V = 8, 256, 768, 4, 128, 30522
D2 = H * F        # 512 concat feature dim
NCORES = 8
VPAD = 30528       # padded vocab (6 zero cols)
NPADC = VPAD - V
LN_EPS = 1e-12
ALPHA = 0.01       # leaky relu slope
MASK_NEG = -50.0   # masked attention logit
NKT = D // 128     # 6 hidden k-tiles for the GAT matmuls
NM = N // 128      # 2 token m-tiles per core
# vocab chunks: [128,1024] psum tiles (2 banks); 30528 = 29*1024 + 832
CHUNKS = [(c0, min(1024, VPAD - c0)) for c0 in range(0, VPAD, 1024)]
NVC = len(CHUNKS)

AX = mybir.AxisListType
AF = mybir.ActivationFunctionType
OP = mybir.AluOpType

_NC_CACHE = {}


def _build(with_ln_b: bool, with_out_b: bool):
    """Build the SPMD Bass program (identical on all 8 cores)."""
    nc = bacc.Bacc(
        "TRN2",
        target_bir_lowering=False,
        debug=False,
        enable_asserts=False,
        num_devices=NCORES,
    )

    # ---- per-core I/O --------------------------------------------------
    xpre = nc.dram_tensor("xpre", [N, D], F32, kind="ExternalInput").ap()
    maskt = nc.dram_tensor("maskt", [N, N], U8, kind="ExternalInput").ap()
    wg = nc.dram_tensor("wg", [D, D2], F16, kind="ExternalInput").ap()
    cpk = nc.dram_tensor("cpk", [D, 2 * H], F16, kind="ExternalInput").ap()
    wst = nc.dram_tensor("wst", [4, 128, VPAD], F16, kind="ExternalInput").ap()
    if with_ln_b:
        brw = nc.dram_tensor("brw", [1, D2], F16, kind="ExternalInput").ap()
        brc = nc.dram_tensor("brc", [1, 2 * H], F16, kind="ExternalInput").ap()
    if with_out_b:
        bvoc = nc.dram_tensor("bvoc", [1, VPAD], F16, kind="ExternalInput").ap()
    out = nc.dram_tensor("out", [N, VPAD], F16, kind="ExternalOutput").ap()

    with tile.TileContext(nc) as tc:
        with (
            tc.tile_pool(name="catf_pool", bufs=1) as catf_pool,
            tc.tile_pool(name="bias_pool", bufs=1) as bias_pool,
        ):
            # local concat features [feature-k, token] per head (== k-tile)
            catf = [
                catf_pool.tile([128, N], F16, tag=f"catf{kt}", name=f"catf{kt}") for kt in range(H)
            ]
            if with_out_b:
                bvoc_sb = bias_pool.tile([1, VPAD], F16, tag="bvoc")
                nc.sync.dma_start(out=bvoc_sb[:], in_=bvoc[:, :])
                ones1v = bias_pool.tile([1, 128], F16, tag="ones1v")
                nc.vector.memset(ones1v[:], 1.0)

            # ==== phase A: embedding LN + GAT (own batch) ==============
            with (
                tc.tile_pool(name="pa", bufs=1) as pa,
                tc.tile_pool(name="pa_tmp", bufs=4) as pa_tmp,
                tc.tile_pool(name="ps_tr", bufs=2, space="PSUM") as ps_tr,
                tc.tile_pool(name="ps_wh", bufs=2, space="PSUM") as ps_wh,
                tc.tile_pool(name="ps_s12", bufs=1, space="PSUM") as ps_s12,
                tc.tile_pool(name="ps_att", bufs=2, space="PSUM") as ps_att,
                tc.tile_pool(name="ps_hp", bufs=1, space="PSUM") as ps_hp,
            ):
                idw = pa.tile([128, 128], F16, tag="idw")
                bass_masks_identity(nc, idw[:])
                negt = pa.tile([128, N], F32, tag="negt")
                nc.vector.memset(negt[:], MASK_NEG)
                ones_nf = pa.tile([1, N], F16, tag="ones_nf")
                nc.vector.memset(ones_nf[:], 1.0)
                eps_sb = pa.tile([128, 1], F32, tag="eps_sb")
                nc.vector.memset(eps_sb[:], LN_EPS)
                if with_ln_b:
                    ones1h = pa.tile([1, 128], F16, tag="ones1h")
                    nc.vector.memset(ones1h[:], 1.0)
                    ones_n = pa.tile([1, N], F16, tag="ones_n")
                    nc.vector.memset(ones_n[:], 1.0)
                    brw_sb = pa.tile([1, D2], F16, tag="brw")
                    nc.sync.dma_start(out=brw_sb[:], in_=brw[:, :])
                    brc_sb = pa.tile([1, 2 * H], F16, tag="brc")
                    nc.sync.dma_start(out=brc_sb[:], in_=brc[:, :])

                # ---- LayerNorm first: xpre DMA must not queue behind
                # the big wst loads (phase B throttles those via its pool)
                xn_sb = [pa.tile([128, D], F16, tag=f"xn{m}", name=f"xn{m}") for m in range(2)]
                xp_t = []
                for m in range(2):
                    xp = pa_tmp.tile([128, D], F32, tag="xp")
                    nc.sync.dma_start(
                        out=xp[:], in_=xpre[m * 128 : (m + 1) * 128, :]
                    )
                    xp_t.append(xp)
                wg_sb = [
                    pa.tile([128, D2], F16, tag=f"wg{kt}", name=f"wg{kt}") for kt in range(NKT)
                ]
                cpk_sb = [
                    pa.tile([128, 2 * H], F16, tag=f"cpk{kt}", name=f"cpk{kt}") for kt in range(NKT)
                ]
                for kt in range(NKT):
                    nc.sync.dma_start(
                        out=wg_sb[kt][:], in_=wg[kt * 128 : (kt + 1) * 128, :]
                    )
                    nc.sync.dma_start(
                        out=cpk_sb[kt][:], in_=cpk[kt * 128 : (kt + 1) * 128, :]
                    )
                mask_sb = [pa.tile([128, N], U8, tag=f"mask{j}", name=f"mask{j}") for j in range(2)]
                for jt in range(2):
                    nc.sync.dma_start(
                        out=mask_sb[jt][:], in_=maskt[jt * 128 : (jt + 1) * 128, :]
                    )
                for m in range(2):
                    xp = xp_t[m]
                    nmu = pa_tmp.tile([128, 1], F32, tag="nmu")
                    nc.vector.tensor_reduce(
                        out=nmu[:], in_=xp[:], axis=AX.X, op=OP.add, negate=True
                    )
                    nc.vector.tensor_scalar_mul(nmu[:], nmu[:], 1.0 / D)
                    xc = pa_tmp.tile([128, D], F32, tag="xc")
                    nc.vector.tensor_scalar_add(xc[:], xp[:], nmu[:, 0:1])
                    sq = pa_tmp.tile([128, D], F32, tag="sq")
                    ssum = pa_tmp.tile([128, 1], F32, tag="ssum")
                    nc.scalar.activation(
                        sq[:], xc[:], AF.Square, accum_out=ssum[:, 0:1]
                    )
                    sd = pa_tmp.tile([128, 1], F32, tag="sd")
                    nc.scalar.activation(
                        sd[:], ssum[:], AF.Sqrt, bias=eps_sb[:, 0:1], scale=1.0 / D
                    )
                    rstd = pa_tmp.tile([128, 1], F32, tag="rstd")
                    nc.vector.reciprocal(rstd[:], sd[:])
                    nc.vector.tensor_scalar_mul(xn_sb[m][:], xc[:], rstd[:, 0:1])

                # ---- transpose xn -> xT[kt] [128 hid, 256 tok] --------
                xt_sb = [pa.tile([128, N], F16, tag=f"xt{kt}", name=f"xt{kt}") for kt in range(NKT)]
                for kt in range(NKT):
                    for m in range(2):
                        ptr = ps_tr.tile([128, 128], F16, tag="ptr")
                        nc.tensor.transpose(
                            ptr[:], xn_sb[m][:, kt * 128 : (kt + 1) * 128], idw[:]
                        )
                        nc.scalar.copy(
                            xt_sb[kt][:, m * 128 : (m + 1) * 128], ptr[:]
                        )

                # ---- fused Wh for all heads: [tokens, 512] ------------
                wh_m = [pa.tile([128, D2], F16, tag=f"whm{m}", name=f"whm{m}") for m in range(2)]
                for m in range(2):
                    pwh = ps_wh.tile([128, D2], F32, tag="pwh")
                    for kt in range(NKT):
                        nc.tensor.matmul(
                            pwh[:],
                            xt_sb[kt][:, m * 128 : (m + 1) * 128],
                            wg_sb[kt][:],
                            start=(kt == 0),
                            stop=(kt == NKT - 1) and not with_ln_b,
                        )
                    if with_ln_b:
                        nc.tensor.matmul(
                            pwh[:], ones1h[:], brw_sb[:], start=False, stop=True
                        )
                    nc.scalar.copy(wh_m[m][:], pwh[:])

                # ---- fused s1/s2 rows: [8 = (h,c), tokens] ------------
                ps12 = ps_s12.tile([2 * H, N], F32, tag="ps12")
                for kt in range(NKT):
                    nc.tensor.matmul(
                        ps12[:],
                        cpk_sb[kt][:],
                        xt_sb[kt][:],
                        start=(kt == 0),
                        stop=(kt == NKT - 1) and not with_ln_b,
                    )
                if with_ln_b:
                    nc.tensor.matmul(
                        ps12[:], brc_sb[:], ones_n[:], start=False, stop=True
                    )
                s12sb = pa.tile([2 * H, N], F16, tag="s12sb")
                nc.scalar.copy(s12sb[:], ps12[:])

                # ---- attention + head output, one head at a time ------
                att = [
                    [pa.tile([128, N], F16, tag=f"att{h}_{m}", name=f"att{h}_{m}") for m in range(2)]
                    for h in range(H)
                ]
                for h in range(H):
                    # rhs rows: [s1(tokens); ones] -- row moves via DMA
                    # (engine ops cannot write at partition offsets != 0)
                    s1r2 = pa_tmp.tile([2, N], F16, tag="s1r2")
                    nc.sync.dma_start(
                        out=s1r2[0:1, :], in_=s12sb[2 * h : 2 * h + 1, :]
                    )
                    nc.sync.dma_start(out=s1r2[1:2, :], in_=ones_nf[0:1, :])
                    for jt in range(2):
                        # lhsT rows: [ones; s2(j-block)]
                        s2st = pa_tmp.tile([2, 128], F16, tag="s2st")
                        nc.sync.dma_start(
                            out=s2st[0:1, :], in_=ones_nf[0:1, 0:128]
                        )
                        nc.sync.dma_start(
                            out=s2st[1:2, :],
                            in_=s12sb[2 * h + 1 : 2 * h + 2, jt * 128 : (jt + 1) * 128],
                        )
                        pet = ps_att.tile([128, N], F32, tag="pet")
                        nc.tensor.matmul(
                            pet[:], s2st[:], s1r2[:], start=True, stop=True
                        )
                        # leakyrelu in 2 ops (only one PSUM input per instr)
                        lr1 = pa_tmp.tile([128, N], F32, tag="lr1")
                        nc.vector.tensor_scalar_mul(lr1[:], pet[:], ALPHA)
                        lr = pa_tmp.tile([128, N], F32, tag="lr")
                        nc.vector.scalar_tensor_tensor(
                            lr[:], lr1[:], 1.0, pet[:], OP.mult, OP.max
                        )
                        nc.vector.copy_predicated(lr[:], mask_sb[jt][:], negt[:])
                        nmax = pa_tmp.tile([128, 1], F32, tag="nmax")
                        nc.vector.tensor_reduce(
                            out=nmax[:], in_=lr[:], axis=AX.X, op=OP.max, negate=True
                        )
                        ex = pa_tmp.tile([128, N], F16, tag="ex")
                        asum = pa_tmp.tile([128, 1], F32, tag="asum")
                        nc.scalar.activation(
                            ex[:], lr[:], AF.Exp, bias=nmax[:, 0:1],
                            accum_out=asum[:, 0:1],
                        )
                        rec = pa_tmp.tile([128, 1], F32, tag="rec")
                        nc.vector.reciprocal(rec[:], asum[:])
                        nc.vector.tensor_scalar_mul(
                            att[h][jt][:], ex[:], rec[:, 0:1]
                        )
                    # hp^T = Wh^T @ att^T, elu -> local catf[h]
                    php = ps_hp.tile([128, N], F32, tag="php")
                    for jt in range(2):
                        nc.tensor.matmul(
                            php[:],
                            wh_m[jt][:, h * 128 : (h + 1) * 128],
                            att[h][jt][:],
                            start=(jt == 0),
                            stop=(jt == 1),
                        )
                    # elu(z) = max(z, min(exp(z)-1, 0))
                    eh = pa_tmp.tile([128, N], F32, tag="eh")
                    nc.scalar.activation(eh[:], php[:], AF.Exp)
                    ddh = pa_tmp.tile([128, N], F16, tag="ddh")
                    nc.vector.tensor_scalar(
                        ddh[:], eh[:], 1.0, 0.0, OP.subtract, OP.min
                    )
                    nc.vector.scalar_tensor_tensor(
                        catf[h][:], php[:], 0.0, ddh[:], OP.add, OP.max
                    )

            # ==== full-vocab output linear + local log_softmax =========
            with (
                tc.tile_pool(name="vp_pool", bufs=1) as vp_pool,
                tc.tile_pool(name="wstream", bufs=3) as wstream,
                tc.tile_pool(name="etmp", bufs=4) as etmp,
                tc.tile_pool(name="stat", bufs=1) as stat,
                tc.tile_pool(name="ps_z", bufs=4, space="PSUM") as ps_z,
            ):
                vp = [
                    vp_pool.tile([128, VPAD], F16, tag=f"vp{m}", name=f"vp{m}") for m in range(NM)
                ]
                # exp sums per 2-chunk group, one tile per m-tile
                NSG = (NVC + 1) // 2
                sums = [
                    stat.tile([128, NSG], F32, tag=f"sums{m}", name=f"sums{m}")
                    for m in range(NM)
                ]
                negone = stat.tile([128, 1], F32, tag="negone")
                nc.vector.memset(negone[:], -1.0)
                pad_sb = stat.tile([128, 1], F32, tag="pad_sb")
                nc.vector.memset(pad_sb[:], NPADC * math.exp(-1.0))

                for ci, (c0, cw) in enumerate(CHUNKS):
                    wc = [
                        wstream.tile([128, 1024], F16, tag=f"wc{kt}", name=f"wc{kt}")
                        for kt in range(4)
                    ]
                    for kt in range(4):
                        nc.sync.dma_start(
                            out=wc[kt][:, 0:cw], in_=wst[kt, :, c0 : c0 + cw]
                        )
                    for m in range(NM):
                        zp = ps_z.tile([128, 1024], F32, tag="z")
                        for s0 in range(0, cw, 512):
                            sw = min(512, cw - s0)
                            for kt in range(4):
                                nc.tensor.matmul(
                                    zp[:, s0 : s0 + sw],
                                    catf[kt][:, m * 128 : (m + 1) * 128],
                                    wc[kt][:, s0 : s0 + sw],
                                    start=(kt == 0),
                                    stop=(kt == 3) and not with_out_b,
                                )
                            if with_out_b:
                                nc.tensor.matmul(
                                    zp[:, s0 : s0 + sw],
                                    ones1v[:],
                                    bvoc_sb[:, c0 + s0 : c0 + s0 + sw],
                                    start=False,
                                    stop=True,
                                )
                        # vp = elu(z) = max(z, min(exp(z)-1, 0))
                        e0 = etmp.tile([128, 1024], F16, tag="e0")
                        nc.scalar.activation(e0[:, 0:cw], zp[:, 0:cw], AF.Exp)
                        dd = etmp.tile([128, 1024], F16, tag="dd")
                        nc.vector.tensor_scalar(
                            dd[:, 0:cw], e0[:, 0:cw], 1.0, 0.0,
                            OP.subtract, OP.min,
                        )
                        nc.vector.scalar_tensor_tensor(
                            vp[m][:, c0 : c0 + cw],
                            zp[:, 0:cw],
                            0.0,
                            dd[:, 0:cw],
                            OP.add,
                            OP.max,
                        )
                    # sum of exp(vp - 1) over each completed 2-chunk group
                    if ci % 2 == 1 or ci == NVC - 1:
                        g0 = (ci // 2) * 2
                        gc0 = CHUNKS[g0][0]
                        gw = c0 + cw - gc0
                        gi = ci // 2
                        for m in range(NM):
                            dum = etmp.tile([128, 2048], F16, tag="dum")
                            nc.scalar.activation(
                                dum[:, 0:gw],
                                vp[m][:, gc0 : gc0 + gw],
                                AF.Exp,
                                bias=negone[:, 0:1],
                                accum_out=sums[m][:, gi : gi + 1],
                            )

                # local logsumexp: pad cols contribute exp(0-1) each;
                # remove, then out = vp - (log(sum) + 1)
                neglog = []
                for m in range(NM):
                    lsum = stat.tile([128, 1], F32, tag=f"lsum{m}", name=f"lsum{m}")
                    nc.vector.tensor_reduce(
                        out=lsum[:], in_=sums[m][:], axis=AX.X, op=OP.add
                    )
                    nc.vector.tensor_scalar_sub(lsum[:], lsum[:], pad_sb[:, 0:1])
                    nl = stat.tile([128, 1], F32, tag=f"neglog{m}", name=f"neglog{m}")
                    nc.scalar.activation(nl[:], lsum[:], AF.Ln)
                    nc.vector.tensor_scalar(
                        nl[:], nl[:], -1.0, 1.0, OP.mult, OP.subtract
                    )
                    neglog.append(nl)
                for m in range(NM):
                    nc.vector.tensor_scalar_add(
                        vp[m][:], vp[m][:], neglog[m][:, 0:1]
                    )
                    nc.sync.dma_start(
                        out=out[m * 128 : (m + 1) * 128, :], in_=vp[m][:]
                    )

    nc.compile()
    return nc


def bass_masks_identity(nc, ident_ap):
    from concourse import masks

    masks.make_identity(nc, ident_ap)


def _host_prep(inputs):
    """Per-core input maps from full inputs (numpy only)."""
    tok = np.asarray(inputs["token_ids"])
    typ = np.asarray(inputs["type_ids"])
    syn = np.asarray(inputs["synset_ids"])
    hw = np.asarray(inputs["highway"]).astype(bool)
    tok_emb = np.asarray(inputs["tok_emb"], dtype=np.float32)
    type_emb = np.asarray(inputs["type_emb"], dtype=np.float32)
    pos_emb = np.asarray(inputs["pos_emb"], dtype=np.float32)
    ln_g = np.asarray(inputs["ln_g"], dtype=np.float32)
    ln_b = np.asarray(inputs["ln_b"], dtype=np.float32)
    W = np.asarray(inputs["W"], dtype=np.float32)
    a = np.asarray(inputs["a"], dtype=np.float32)
    out_W = np.asarray(inputs["out_W"], dtype=np.float32)
    out_b = np.asarray(inputs["out_b"], dtype=np.float32)

    # embeddings (host gather + add, f32 like the reference)
    x_pre = tok_emb[tok] + type_emb[typ] + pos_emb[:N][None]  # (B,N,D)

    # graph mask (host index logic), transposed to [j, i], 1 = masked-out
    vis = syn[:, :, None] == syn[:, None, :]
    s1m = (typ == 1) & hw
    s3m = (typ == 3) & hw
    d1 = np.isin(typ, [0, 2, 5]) & hw
    d3 = np.isin(typ, [6, 4, 0]) & hw
    vis = vis | (s1m[:, :, None] & d1[:, None, :]) | (s3m[:, :, None] & d3[:, None, :])
    mask = vis & (tok != 0)[:, None, :]  # (B,N,N) over [i,j]
    maskt = (~mask).transpose(0, 2, 1).astype(np.uint8)  # (B,N,N) over [j,i]

    # GAT weights: fold ln_g; head-major [D, H*F] plus c1/c2 contraction cols
    Wg = W * ln_g[None, :, None]  # (H,D,F)
    wg = Wg.transpose(1, 0, 2).reshape(D, D2).astype(np.float16)
    a1, a2 = a[:, :F], a[:, F:]
    c1 = np.einsum("hdf,hf->hd", Wg, a1)  # (H,D)
    c2 = np.einsum("hdf,hf->hd", Wg, a2)
    cpk = np.stack([c1, c2], axis=1).reshape(2 * H, D).T  # (D, 2H) cols (h,c)
    cpk = np.ascontiguousarray(cpk).astype(np.float16)

    with_ln_b = bool(np.any(ln_b != 0.0))
    brw = brc = None
    if with_ln_b:
        brw = (ln_b @ W.transpose(1, 0, 2).reshape(D, D2)).reshape(1, D2)
        brw = brw.astype(np.float16)
        b1 = np.einsum("hdf,hf->hd", W, a1)
        b2 = np.einsum("hdf,hf->hd", W, a2)
        brc = (ln_b @ np.stack([b1, b2], axis=1).reshape(2 * H, D).T).reshape(1, 2 * H)
        brc = brc.astype(np.float16)

    # full padded out_W^T [512, 30528] -> [4, 128, VPAD] (same on all cores)
    wpad = np.zeros((VPAD, D2), dtype=np.float32)
    wpad[:V] = out_W
    wst = np.ascontiguousarray(wpad.T.astype(np.float16).reshape(4, 128, VPAD))
    with_out_b = bool(np.any(out_b != 0.0))
    bvocp = None
    if with_out_b:
        bpad = np.zeros((VPAD,), dtype=np.float32)
        bpad[:V] = out_b
        bvocp = np.ascontiguousarray(bpad.reshape(1, VPAD).astype(np.float16))

    in_maps = []
    for c in range(NCORES):
        m = {
            "xpre": np.ascontiguousarray(x_pre[c]),
            "maskt": np.ascontiguousarray(maskt[c]),
            "wg": wg,
            "cpk": cpk,
            "wst": wst,
        }
        if with_ln_b:
            m["brw"] = brw
            m["brc"] = brc
        if with_out_b:
            m["bvoc"] = bvocp
        in_maps.append(m)
    return in_maps, with_ln_b, with_out_b


def kernel(**inputs) -> np.ndarray:
    in_maps, with_ln_b, with_out_b = _host_prep(inputs)

    key = (with_ln_b, with_out_b)
    if key not in _NC_CACHE:
        _NC_CACHE[key] = _build(with_ln_b, with_out_b)
    nc = _NC_CACHE[key]

    trace = bool(int(os.environ.get("KBERT_TRACE", "0")))
    res = run_bass_kernel_spmd(
        nc, in_maps, core_ids=list(range(NCORES)), trace=trace
    )
    if trace and res.exec_time_ns is not None:
        print(f"HW exec time: {res.exec_time_ns} ns")
        if res.instructions_and_trace is not None:
            print(f"trace: {res.instructions_and_trace[1]}")

    full = np.empty((B, N, V), dtype=np.float32)
    for c in range(NCORES):
        full[c] = res.results[c]["out"][:, :V]
    return full
